# revision 46
# baseline (speedup 1.0000x reference)
"""Trainium2 Bass kernel for LoFTR-style encoder layer (sparse attention + convs).

Sharding: pure data-parallel over batch B=8 -> 8 NeuronCores (one batch
element per core). BN statistics are all-reduced across cores.

Schedule: ft is uploaded pre-cast to bf16 (halves the input DMA and
removes the on-device casts) with the first-needed slices leading both
DMA queues, so the K/V projections and conv1 start within ~10us;
conv1's row-tiles are interleaved with the vector-bound attention
phase (f-only tiles have no message dependency; later tiles follow one
message-tile behind) so the PE never starves. Each BN barrier uses a
two-chunk all-reduce: chunk A (most tiles) launches while the PE still
has tiles in flight and hides its full latency (including cross-core
drift); only chunk B's short latency is exposed. Two dummy warm-up
collectives absorb the CC cold-start during the attention window. BN1
is folded into conv2 (weights scaled by scl1 in o=0-first quarters,
bias from host-precomputed tap-summed weights, padding halo written as
-sh1/scl1 so zero-padding is reproduced exactly; halo writes split
vector/pool). conv2 is tiled at 6 rows per psum bank (fewer weight
loads); the last tile's y1p/y2 writes are deferred past the AR-B
launch. The BN2+residual tail runs on three engines concurrently
(scalar scale-shift in place, vector out-of-place, pool residual
adds), each chunk DMA-ing out immediately on the two hardware DGE
queues. Output is stored bf16 and converted on host.

Device layout is channel-major ([C, spatial]); host does the (free)
transposes / weight reordering / bf16 casts when staging inputs, and
transposes the per-core outputs back.
"""

import os
import sys

import numpy as np

for _p in ("/opt/trn_rl_repo", os.path.expanduser("~/.axon_site/_ro/trn_rl_repo")):
    if os.path.isdir(_p) and _p not in sys.path:
        sys.path.insert(0, _p)

import ml_dtypes

import concourse.bass as bass
import concourse.mybir as mybir
import concourse.tile as tile
from concourse import bacc
from concourse.bass_utils import run_bass_kernel_spmd

F32 = mybir.dt.float32
BF16 = mybir.dt.bfloat16
AF = mybir.ActivationFunctionType
ALU = mybir.AluOpType

NCORES = 8
H = W = 80
HW = H * W          # 6400
D = 256
NI = 3200           # inside positions (image rows 0..39)
NHEAD = 8
PW = W + 2          # 82 padded width
BN_EPS = 1e-5
BN_N = float(NCORES * HW)

# conv row-tiling: 5 output rows per psum tile -> N = 5*82 = 410 <= 512
RT = 5
NRT = H // RT       # 16
NT = RT * PW        # 410

LAST_EXEC_NS = None
LAST_MEAN_EXEC_NS = None

_cache = {}


def _bd(ap3):
    return ap3.rearrange("p a b -> p (a b)")


def _r3(ap2, a):
    return ap2.rearrange("p (a b) -> p a b", a=a)


def build_nc():
    nc = bacc.Bacc(
        "TRN2", target_bir_lowering=False, debug=False, num_devices=NCORES
    )

    ft_d = nc.dram_tensor("ft", [D, HW], BF16, kind="ExternalInput")
    wqt_d = nc.dram_tensor("wqt", [128, 2, D], BF16, kind="ExternalInput")
    wkt_d = nc.dram_tensor("wkt", [128, 2, D], BF16, kind="ExternalInput")
    wvt_d = nc.dram_tensor("wvt", [128, 2, D], BF16, kind="ExternalInput")
    c1w_d = nc.dram_tensor("c1w", [128, 36, D], BF16, kind="ExternalInput")
    c2w_d = nc.dram_tensor("c2w", [128, 18, D], BF16, kind="ExternalInput")
    ws2_d = nc.dram_tensor("ws2", [128, 2, D], BF16, kind="ExternalInput")
    bn1g_d = nc.dram_tensor("bn1g", [D, 1], F32, kind="ExternalInput")
    bn1b_d = nc.dram_tensor("bn1b", [D, 1], F32, kind="ExternalInput")
    bn2g_d = nc.dram_tensor("bn2g", [D, 1], F32, kind="ExternalInput")
    bn2b_d = nc.dram_tensor("bn2b", [D, 1], F32, kind="ExternalInput")
    mblk_d = nc.dram_tensor("mblk", [8, 256], BF16, kind="ExternalInput")
    out_d = nc.dram_tensor("out_t", [D, HW], BF16, kind="ExternalOutput")

    groups = [list(range(NCORES))]

    with tile.TileContext(nc) as tc:
        with (
            tc.tile_pool(name="pers", bufs=1) as pers,
            tc.tile_pool(name="bigp", bufs=2) as bigp,
            tc.tile_pool(name="ftio", bufs=2) as ftio,
            tc.tile_pool(name="qtp", bufs=4) as qtp,
            tc.tile_pool(name="scr", bufs=4) as scr,
            tc.tile_pool(name="small", bufs=1) as small,
            tc.tile_pool(name="fin", bufs=4) as fin,
            tc.tile_pool(name="psA", bufs=4, space="PSUM") as psA,
            tc.tile_pool(name="psS", bufs=2, space="PSUM") as psS,
            tc.tile_pool(name="psC", bufs=2, space="PSUM") as psC,
            tc.tile_pool(name="dram", bufs=1, space="DRAM") as dramp,
        ):
            # ---------------- weights / consts ------------------------------
            # ft arrives pre-cast to bf16 (host does the transpose+cast for
            # free), DMA'd straight into the persistent ftb tiles across four
            # trigger queues (sync/gpsimd for ft, scalar/vector for conv
            # weights) so the K/V matmuls and conv1 start as early as the DMA
            # engines allow.
            wqt = pers.tile([128, 2, D], BF16, tag="wqt", name="wqt")
            wkt = pers.tile([128, 2, D], BF16, tag="wkt", name="wkt")
            wvt = pers.tile([128, 2, D], BF16, tag="wvt", name="wvt")
            ftb = [
                pers.tile([128, HW], BF16, tag=f"ftb{m}", name=f"ftb{m}")
                for m in range(2)
            ]
            ip1 = [
                pers.tile([128, 84, PW], BF16, tag=f"ip1_{c}", name=f"ip1_{c}")
                for c in range(4)
            ]
            ip1f = [_bd(ip1[c][:, :, :]) for c in range(4)]
            c1w = pers.tile([128, 36, D], BF16, tag="c1w", name="c1w")
            c2w = pers.tile([128, 18, D], BF16, tag="c2w", name="c2w")
            ws2 = pers.tile([128, 2, D], BF16, tag="ws2", name="ws2")
            maskblk = pers.tile([8, 256], BF16, tag="maskblk", name="maskblk")

            # ft half 0 rides the sync hw-DGE queue; half 1 rides gpsimd's
            # software-DGE queue. gpsimd descriptor generation is slower
            # (~2.5us/transfer) but paces the HBM traffic deterministically —
            # with both hw queues blasting at once, per-core completion
            # jitter (~15us) lands in the exposed BN all-reduce waits.
            def ft_dma(m, c0, c1):
                eng = nc.sync if m == 0 else nc.gpsimd
                eng.dma_start(ftb[m][:, c0:c1],
                              ft_d[m * 128 : (m + 1) * 128, c0:c1])

            ft_dma(0, 0, 256)
            ft_dma(1, 0, 256)
            nc.sync.dma_start(wkt[:, :, :], wkt_d[:, :, :])
            nc.sync.dma_start(wvt[:, :, :], wvt_d[:, :, :])
            ft_dma(0, 256, 1600)
            ft_dma(1, 256, 1600)
            ft_dma(0, 1600, 3200)
            ft_dma(1, 1600, 3200)
            nc.sync.dma_start(wqt[:, :, :], wqt_d[:, :, :])
            nc.scalar.dma_start(c1w[:, 0:18, :], c1w_d[:, 0:18, :])
            nc.scalar.dma_start(c1w[:, 18:36, :], c1w_d[:, 18:36, :])
            ft_dma(0, 3200, 4800)
            ft_dma(1, 3200, 4800)
            ft_dma(0, 4800, 6400)
            ft_dma(1, 4800, 6400)
            nc.scalar.dma_start(c2w[:, :, :], c2w_d[:, :, :])
            nc.sync.dma_start(ws2[:, :, :], ws2_d[:, :, :])
            nc.sync.dma_start(maskblk[:, :], mblk_d[:, :])

            # warm up the CC cores with two dummy all-reduces: the first two
            # collective executions pay a ~20us cold-start which would
            # otherwise land in the BN1 bubble; here it overlaps attention.
            wrm = small.tile([8, 2], F32, tag="wrm", name="wrm")
            nc.vector.memset(wrm[:, :], 0.0)
            for w in range(2):
                win = dramp.tile([8, 2], F32, tag=f"win{w}", name=f"win{w}")
                wout = dramp.tile([8, 2], F32, tag=f"wout{w}", name=f"wout{w}")
                nc.sync.dma_start(win[:, :], wrm[:, :])
                nc.gpsimd.collective_compute(
                    "AllReduce", ALU.add, replica_groups=groups,
                    ins=[win[:, :].opt()], outs=[wout[:, :].opt()],
                )

            eps_t = small.tile([128, 1], F32, tag="eps_t", name="eps_t")
            nc.vector.memset(eps_t[:, :], BN_EPS)
            ones_t = small.tile([128, 164], F32, tag="ones_t", name="ones_t")
            nc.vector.memset(ones_t[:, :], 1.0)
            g1 = small.tile([128, 2], F32, tag="g1", name="g1")
            b1 = small.tile([128, 2], F32, tag="b1", name="b1")
            g2 = small.tile([128, 2], F32, tag="g2", name="g2")
            b2 = small.tile([128, 2], F32, tag="b2", name="b2")
            for o in range(2):
                sl = slice(o * 128, (o + 1) * 128)
                nc.sync.dma_start(g1[:, o : o + 1], bn1g_d[sl, :])
                nc.sync.dma_start(b1[:, o : o + 1], bn1b_d[sl, :])
                nc.sync.dma_start(g2[:, o : o + 1], bn2g_d[sl, :])
                nc.sync.dma_start(b2[:, o : o + 1], bn2b_d[sl, :])

            # ---------------- ip1 halo zeroing (targeted, not full tiles) ---
            # tile row r+2 == image row r; taps read tile rows 1..82 and the
            # 410-window spills touch rows 0 and 83 in discarded columns.
            for c in range(2):  # f-chunks
                nc.vector.memset(ip1[c][:, 0:2, :], 0.0)
                nc.vector.memset(ip1[c][:, 82:84, :], 0.0)
                nc.vector.memset(ip1[c][:, 2:82, 0:1], 0.0)
                nc.vector.memset(ip1[c][:, 2:82, 81:82], 0.0)
            for c in range(2, 4):  # t-chunks: zero band above message rows
                nc.vector.memset(ip1[c][:, 34:42, :], 0.0)
                nc.vector.memset(ip1[c][:, 82:84, :], 0.0)
                nc.vector.memset(ip1[c][:, 42:82, 0:1], 0.0)
                nc.vector.memset(ip1[c][:, 42:82, 81:82], 0.0)

            # ---------------- per-segment cast + ip1 interior + K/V proj ----
            ke = bigp.tile([128, 25, D], BF16, tag="big", name="ke")
            # ve layout [ones, v0..v255, ones]: per half m the 129 columns
            # [m*129 : m*129+129) are contiguous = [ones|v_m] or [v_m|ones]
            ve = bigp.tile([128, 25, D + 2], BF16, tag="big", name="ve")
            nc.vector.memset(ve[:, :, 0:1], 1.0)
            nc.vector.memset(ve[:, :, 257:258], 1.0)

            def kv_tile(i):
                ps = psA.tile([128, D], F32, tag="psA", name="psA")
                for ki in range(2):
                    nc.tensor.matmul(
                        ps[:, :],
                        ftb[ki][:, i * 128 : (i + 1) * 128],
                        wkt[:, ki, :],
                        start=(ki == 0),
                        stop=(ki == 1),
                    )
                # elu(x)+1 = relu(x) + exp(min(x,0))
                sm = scr.tile([128, 400], F32, tag="scr", name="sm")
                se = scr.tile([128, 400], F32, tag="scr", name="se")
                nc.vector.tensor_scalar_min(sm[:, :D], ps[:, :], 0.0)
                nc.scalar.activation(se[:, :D], sm[:, :D], AF.Exp)
                nc.vector.scalar_tensor_tensor(
                    ke[:, i, :], ps[:, :], 0.0, se[:, :D], ALU.max, ALU.add
                )
                ps2 = psA.tile([128, D], F32, tag="psA", name="psA")
                for ki in range(2):
                    nc.tensor.matmul(
                        ps2[:, :],
                        ftb[ki][:, i * 128 : (i + 1) * 128],
                        wvt[:, ki, :],
                        start=(ki == 0),
                        stop=(ki == 1),
                    )
                nc.vector.tensor_copy(ve[:, i, 1:257], ps2[:, :])

            def stage_seg(s, m):
                fseg = ftb[m][:, s * 1600 : (s + 1) * 1600]
                nc.vector.tensor_copy(
                    ip1[m][:, 2 + 20 * s : 22 + 20 * s, 1:81], _r3(fseg, 20)
                )

            kv_tile(0)
            kv_tile(1)
            for s in range(2):  # inside segments
                for m in range(2):
                    stage_seg(s, m)
                for i in range(12 * s + 2 * (1 - s), 12 * s + 12 + s):
                    kv_tile(i)  # s=0: 2..11, s=1: 12..24

            for s in range(2, 4):  # outside segments
                for m in range(2):
                    stage_seg(s, m)

            # ---------------- conv1 helpers ----------------
            y1p = [
                pers.tile([128, 84, PW], BF16, tag=f"y1p_{o}", name=f"y1p_{o}")
                for o in range(2)
            ]
            for o in range(2):  # spill-read guard rows (discarded columns)
                nc.vector.memset(y1p[o][:, 0:1, :], 0.0)
                nc.vector.memset(y1p[o][:, 83:84, :], 0.0)
            # BN1 stats are all-reduced in two chunks: chunk A (tiles 0..10)
            # launches mid-attention and hides its full latency under the
            # remaining conv1 tiles; only chunk B's short latency is exposed.
            N1A = 11
            stats1 = small.tile([128, 4 * N1A], F32, tag="stats1", name="stats1")
            stats1b = small.tile([128, 4 * (NRT - N1A)], F32, tag="stats1b",
                                 name="stats1b")

            def conv1_tile(j):
                r0 = RT * j
                if j < N1A:
                    st, jj, ncol = stats1, j, N1A
                else:
                    st, jj, ncol = stats1b, j - N1A, NRT - N1A
                fast = j >= 14  # last tiles: stats first, y1p copy on vector
                taps = []
                for c in range(4):
                    for ky in range(3):
                        if c >= 2 and r0 + ky + 4 < 41:
                            continue  # t-channel rows all zero
                        for kx in range(3):
                            taps.append((c, ky, kx))
                for o in range(2):
                    ps = psC.tile([128, NT], F32, tag="psC", name="psC")
                    for idx, (c, ky, kx) in enumerate(taps):
                        s = (r0 + ky + 1) * PW + kx - 1
                        nc.tensor.matmul(
                            ps[:, :],
                            c1w[:, (ky * 3 + kx) * 4 + c, o * 128 : (o + 1) * 128],
                            ip1f[c][:, s : s + NT],
                            start=(idx == 0),
                            stop=(idx == len(taps) - 1),
                        )
                    val = _r3(ps[:, :], RT)[:, :, 1:81]
                    sq = scr.tile([128, 400], F32, tag="scr", name="sq")
                    if fast:
                        nc.scalar.activation(
                            _r3(sq[:, :], RT), val, AF.Square,
                            accum_out=st[:, (o * 2 + 1) * ncol + jj :
                                         (o * 2 + 1) * ncol + jj + 1],
                        )
                        nc.vector.tensor_reduce(
                            st[:, (o * 2) * ncol + jj : (o * 2) * ncol + jj + 1],
                            val, mybir.AxisListType.XY, ALU.add,
                        )
                        if j == NRT - 1:
                            # last tile: y1p copy deferred past the AR-B
                            # launch so the collective triggers ASAP
                            defer1.append((o, r0, val))
                        else:
                            # scalar (after the Square) keeps the vector
                            # queue clear for the AR-B reduce
                            nc.scalar.copy(
                                y1p[o][:, 2 + r0 : 7 + r0, 1:81], val
                            )
                        continue
                    nc.scalar.copy(
                        y1p[o][:, 2 + r0 : 7 + r0, 1:81], val
                    )
                    nc.vector.tensor_reduce(
                        st[:, (o * 2) * ncol + jj : (o * 2) * ncol + jj + 1],
                        val, mybir.AxisListType.XY, ALU.add,
                    )
                    nc.scalar.activation(
                        _r3(sq[:, :], RT), val, AF.Square,
                        accum_out=st[:, (o * 2 + 1) * ncol + jj :
                                     (o * 2 + 1) * ncol + jj + 1],
                    )

            # f-only conv1 tiles 0,1 ahead of the KV->bd accumulation:
            # the bd chain waits on per-i scalar ve copies, so give the PE
            # filler work to absorb that lag.
            conv1_tile(0)
            conv1_tile(1)

            # ---------------- KV + Ksum -> block-diag BD ----------------
            bd = [
                pers.tile([128, 136], BF16, tag=f"bd{m}", name=f"bd{m}")
                for m in range(2)
            ]
            for m in range(2):
                psm = psA.tile([128, 129], F32, tag="psA", name="psA")
                for i in range(25):
                    nc.tensor.matmul(
                        psm[:, :],
                        ke[:, i, m * 128 : (m + 1) * 128],
                        ve[:, i, m * 129 : m * 129 + 129],
                        start=(i == 0),
                        stop=(i == 24),
                    )
                kcol = 0 if m == 0 else 128
                voff = 1 - m
                nc.vector.memset(bd[m][:, :], 0.0)
                for hh in range(4):
                    h = m * 4 + hh
                    lh = hh * 32
                    nc.vector.tensor_copy(
                        bd[m][lh : lh + 32, lh : lh + 32],
                        psm[lh : lh + 32, voff + lh : voff + lh + 32],
                    )
                    nc.vector.tensor_copy(
                        bd[m][lh : lh + 32, 128 + h : 129 + h],
                        psm[lh : lh + 32, kcol : kcol + 1],
                    )

            # ---------- Q/message pipeline interleaved with f-only conv1 ----
            def qproj(j):
                lsl = slice(NI + j * 400, NI + (j + 1) * 400)
                qt = [
                    qtp.tile([128, 400], BF16, tag="qteT", name=f"qt{m}")
                    for m in range(2)
                ]
                qps = []
                for m in range(2):
                    ps = psA.tile([128, 400], F32, tag="psA", name="psA")
                    for ki in range(2):
                        nc.tensor.matmul(
                            ps[:, :],
                            wqt[:, ki, m * 128 : (m + 1) * 128],
                            ftb[ki][:, lsl],
                            start=(ki == 0),
                            stop=(ki == 1),
                        )
                    qps.append(ps)
                return qt, qps

            def qelu(qt, qps):
                for m in range(2):
                    sm = scr.tile([128, 400], F32, tag="scr", name="smq")
                    se = scr.tile([128, 400], F32, tag="scr", name="seq")
                    nc.vector.tensor_scalar_min(sm[:, :], qps[m][:, :], 0.0)
                    nc.scalar.activation(se[:, :], sm[:, :], AF.Exp)
                    nc.vector.scalar_tensor_tensor(
                        qt[m][:, :], qps[m][:, :], 0.0, se[:, :], ALU.max, ALU.add
                    )

            def qmessage(j, qt):
                pss = psS.tile([8, 400], F32, tag="psS", name="psS")
                for ki in range(2):
                    nc.tensor.matmul(
                        pss[:, :],
                        bd[ki][:, 128:136],
                        qt[ki][:, :],
                        start=(ki == 0),
                        stop=(ki == 1),
                    )
                # S >> eps (S >= ~1e2), so 1/(S+eps) == 1/S in fp32
                rsf = scr.tile([128, 400], F32, tag="scr", name="rsf")
                rs = scr.tile([128, 400], BF16, tag="scr", name="rs")
                nc.vector.reciprocal_approx_fast(rsf[:8, :], pss[:, :])
                nc.scalar.copy(rs[:8, :], rsf[:8, :])

                for m in range(2):
                    psg = psA.tile([128, 400], F32, tag="psA", name="psA")
                    nc.tensor.matmul(
                        psg[:, :], bd[m][:, 0:128], qt[m][:, :],
                        start=True, stop=True,
                    )
                    pre = psS.tile([128, 400], F32, tag="psS", name="psS")
                    nc.tensor.matmul(
                        pre[:, :], maskblk[:, m * 128 : (m + 1) * 128], rs[:8, :]
                    )
                    preb = scr.tile([128, 400], BF16, tag="scr", name="preb")
                    nc.scalar.copy(preb[:, :], pre[:, :])
                    # l-tile j = image rows 40+5j..44+5j -> tile rows 42+5j..
                    nc.vector.tensor_tensor(
                        ip1[2 + m][:, 42 + 5 * j : 47 + 5 * j, 1:81],
                        _r3(psg[:, :], RT),
                        _r3(preb[:, :], RT),
                        ALU.mult,
                    )

            def ar_chunk(stats, ncol, ngrp, tag):
                sv = stats[:, :].rearrange("p (k j) -> p k j", j=ncol)
                bnst = small.tile([128, ngrp], F32, tag=f"bnst{tag}",
                                  name=f"bnst{tag}")
                arin = dramp.tile([128, ngrp], F32, tag=f"arin{tag}",
                                  name=f"arin{tag}")
                arout = dramp.tile([128, ngrp], F32, tag=f"arout{tag}",
                                   name=f"arout{tag}")
                nc.vector.tensor_reduce(
                    bnst[:, :], sv[:, :, :], mybir.AxisListType.X, ALU.add
                )
                return bnst, arin, arout

            def ar_launch(bnst, arin, arout):
                nc.sync.dma_start(arin[:, :], bnst[:, :])
                nc.gpsimd.collective_compute(
                    "AllReduce", ALU.add, replica_groups=groups,
                    ins=[arin[:, :].opt()], outs=[arout[:, :].opt()],
                )

            def ar_fetch(arout, tag, ngrp=4):
                g = small.tile([128, ngrp], F32, tag=f"gst{tag}",
                               name=f"gst{tag}")
                nc.sync.dma_start(g[:, :], arout[:, :])
                return g

            # software pipeline: Qproj j+1 is issued before the dependent
            # attention tail of j; f-only conv1 tiles fill the PE while the
            # vector engine runs elu/Z.
            ar1A = None
            qt_c, qps_c = qproj(0)
            for j in range(8):
                qelu(qt_c, qps_c)
                if j <= 4:
                    conv1_tile(j + 2)   # f-only tiles (t rows all zero)
                nxt = qproj(j + 1) if j < 7 else None
                qmessage(j, qt_c)
                if j >= 1:
                    # tile j+6 needs message j-1 (just written): keeps the PE
                    # queue deep so the elu chain latency never starves it
                    conv1_tile(j + 6)
                if j == 4:
                    # stats chunk A (tiles 0..10) all-reduce: ~60us of PE
                    # work remains, so its full latency (incl. core skew)
                    # hides under conv1.
                    ar1A = ar_chunk(stats1, N1A, 4, "1a")
                    ar_launch(*ar1A)
                if nxt is not None:
                    qt_c, qps_c = nxt
            gst1a = ar_fetch(ar1A[2], "1a")

            # pre-warm the sqrt activation table now that the last Exp is
            # issued: the 1.28us table switch hides under conv1 instead of
            # landing in the BN1 critical path.
            sqwrm = small.tile([128, 1], F32, tag="sqwrm", name="sqwrm")
            nc.scalar.activation(sqwrm[:, :], eps_t[:, :], AF.Sqrt)

            defer1 = []
            for j in range(14, NRT):
                conv1_tile(j)
            ar1B = ar_chunk(stats1b, NRT - N1A, 4, "1b")
            ar_launch(*ar1B)
            # the deferred copies drain on the idle vector queue during the
            # AR-B wait, well before conv2's first psum reuses their banks
            for o, r0, val in defer1:
                nc.vector.tensor_copy(y1p[o][:, 2 + r0 : 7 + r0, 1:81], val)
            gst1b = ar_fetch(ar1B[2], "1b")
            gst1 = small.tile([128, 4], F32, tag="gst1", name="gst1")
            nc.vector.tensor_tensor(gst1[:, :], gst1a[:, :], gst1b[:, :],
                                    ALU.add)

            def bn_coeffs(gst, gg, bb, tag, no=2):
                nm = small.tile([128, no], F32, tag=f"nm{tag}", name=f"nm{tag}")
                ex2 = small.tile([128, no], F32, tag=f"ex2{tag}", name=f"ex2{tag}")
                var = small.tile([128, no], F32, tag=f"var{tag}", name=f"var{tag}")
                sd = small.tile([128, no], F32, tag=f"sd{tag}", name=f"sd{tag}")
                rsd = small.tile([128, no], F32, tag=f"rsd{tag}", name=f"rsd{tag}")
                scl = small.tile([128, no], F32, tag=f"scl{tag}", name=f"scl{tag}")
                sh = small.tile([128, no], F32, tag=f"sh{tag}", name=f"sh{tag}")
                gv = gst[:, :].rearrange("p (o k) -> p k o", k=2)
                nc.vector.tensor_scalar_mul(nm[:, :], gv[:, 0, :], -1.0 / BN_N)
                nc.vector.tensor_scalar_mul(ex2[:, :], gv[:, 1, :], 1.0 / BN_N)
                # var_neg = m^2 - E[x^2];  sd = sqrt(-var_neg + eps)
                nc.vector.tensor_tensor(var[:, :], nm[:, :], nm[:, :], ALU.mult)
                nc.vector.tensor_tensor(
                    var[:, :], var[:, :], ex2[:, :], ALU.subtract
                )
                nc.scalar.activation(
                    sd[:, :], var[:, :], AF.Sqrt, bias=eps_t[:, 0:1], scale=-1.0
                )
                nc.vector.reciprocal(rsd[:, :], sd[:, :])
                nc.vector.tensor_tensor(scl[:, :], rsd[:, :], gg[:, :], ALU.mult)
                nc.vector.tensor_tensor(sh[:, :], nm[:, :], scl[:, :], ALU.mult)
                nc.vector.tensor_tensor(sh[:, :], sh[:, :], bb[:, :], ALU.add)
                return scl, sh

            scl1, sh1 = bn_coeffs(gst1, g1, b1, "1")

            # fold BN1 into conv2: w2' = w2 * scl1[c]; halo = -sh1/scl1 so
            # zero-padding maps to BN-output zero; bias2[o] = sum_{c,k} w2*sh1
            # The o=0 weight quarters are scaled first (conv2 tile 0 starts
            # on them) and the y1p[1] halo writes run on the pool engine in
            # parallel with vector's y1p[0] halos.
            hv1 = small.tile([128, 2], F32, tag="hv1", name="hv1")
            rscl = small.tile([128, 2], F32, tag="rscl", name="rscl")
            nc.vector.reciprocal(rscl[:, :], scl1[:, :])
            nc.vector.scalar_tensor_tensor(
                hv1[:, :], sh1[:, :], -1.0, rscl[:, :], ALU.mult, ALU.mult
            )

            def halos(o, eng):
                hvo = hv1[:, o : o + 1]
                eng.tensor_scalar(
                    y1p[o][:, 1:2, :], _r3(ones_t[:, 0:82], 1), hvo, None,
                    ALU.mult,
                )
                eng.tensor_scalar(
                    y1p[o][:, 82:83, :], _r3(ones_t[:, 0:82], 1), hvo, None,
                    ALU.mult,
                )
                eng.tensor_scalar(
                    y1p[o][:, 2:82, 0:1], _r3(ones_t[:, 0:80], 80), hvo, None,
                    ALU.mult,
                )
                eng.tensor_scalar(
                    y1p[o][:, 2:82, 81:82], _r3(ones_t[:, 0:80], 80), hvo,
                    None, ALU.mult,
                )

            halos(1, nc.gpsimd)
            c2wv = c2w[:, :, :].rearrange("p (t c) o -> p t c o", c=2)
            for oh in range(2):
                if oh == 1:
                    halos(0, nc.vector)
                for ck in range(2):
                    nc.vector.tensor_scalar(
                        c2wv[:, :, ck, oh * 128 : (oh + 1) * 128],
                        c2wv[:, :, ck, oh * 128 : (oh + 1) * 128],
                        scl1[:, ck : ck + 1], None, ALU.mult,
                    )


            bias2 = small.tile([128, 2], F32, tag="bias2", name="bias2")
            sh1b = small.tile([128, 2], BF16, tag="sh1b", name="sh1b")
            nc.scalar.copy(sh1b[:, :], sh1[:, :])
            for o in range(2):
                psb = psC.tile([128, NT], F32, tag="psC", name="psC")
                for ck in range(2):
                    nc.tensor.matmul(
                        psb[:, 0:1],
                        ws2[:, ck, o * 128 : (o + 1) * 128],
                        sh1b[:, ck : ck + 1],
                        start=(ck == 0),
                        stop=(ck == 1),
                    )
                nc.scalar.copy(bias2[:, o : o + 1], psb[:, 0:1])

            # ---------------- conv2 (+ stats), o-phased ----------------
            # The two output-channel halves have independent BN statistics:
            # all o=0 tiles run first, so their all-reduce, coefficients,
            # BN-apply, residual add and output DMA all hide under the o=1
            # tile compute; only the o=1 half's chunk-B collective and
            # 4-chunk apply are exposed at the end. Tiled at 6 rows per
            # psum bank (13 full tiles + one 2-row tail).
            y2 = [
                bigp.tile([128, HW], BF16, tag="big", name=f"y2_{o}")
                for o in range(2)
            ]
            RT2 = 6
            NRT2 = 14
            N2A = 11
            st_o0 = small.tile([128, 2 * NRT2], F32, tag="st_o0", name="st_o0")
            stats2 = small.tile([128, 2 * N2A], F32, tag="stats2", name="stats2")
            stats2b = small.tile([128, 2 * (NRT2 - N2A)], F32, tag="stats2b",
                                 name="stats2b")
            y1pf = [_bd(y1p[c][:, :, :]) for c in range(2)]

            def bias_fix(bnst, o, npos):
                # reduce() summed raw psum values; the true sums need
                # +npos*bias2 (the sq stats were already biased); rides a
                # hidden chunk, off the critical path.
                nc.vector.scalar_tensor_tensor(
                    bnst[:, 0:1], bias2[:, o : o + 1],
                    float(npos), bnst[:, 0:1], ALU.mult, ALU.add
                )

            defer2 = []

            def conv2_half(j, o, st, jj, ncol, fast):
                r0 = RT2 * j
                rows = RT2 if j < NRT2 - 1 else H - RT2 * (NRT2 - 1)
                nt = rows * PW
                ysl = slice(r0 * 80, (r0 + rows) * 80)
                ps = psC.tile([128, 492], F32, tag="psC", name="psC")
                idx = 0
                for c in range(2):
                    for ky in range(3):
                        for kx in range(3):
                            s = (r0 + ky + 1) * PW + kx - 1
                            nc.tensor.matmul(
                                ps[:, 0:nt],
                                c2w[:, (ky * 3 + kx) * 2 + c,
                                    o * 128 : (o + 1) * 128],
                                y1pf[c][:, s : s + nt],
                                start=(idx == 0),
                                stop=(idx == 17),
                            )
                            idx += 1
                val = _r3(ps[:, 0:nt], rows)[:, :, 1:81]
                sq = scr.tile([128, 496], F32, tag="scr2", name="sq2")
                if fast:
                    # stats first; the y2 write trails on scalar (or defers
                    # past the AR-B launch for the last tile)
                    nc.scalar.activation(
                        _r3(sq[:, 0 : rows * 80], rows), val, AF.Square,
                        bias=bias2[:, o : o + 1],
                        accum_out=st[:, ncol + jj : ncol + jj + 1],
                    )
                    nc.vector.tensor_reduce(
                        st[:, jj : jj + 1], val,
                        mybir.AxisListType.XY, ALU.add,
                    )
                    if j == NRT2 - 1:
                        defer2.append((o, ysl, rows, val))
                    else:
                        nc.scalar.activation(
                            _r3(y2[o][:, ysl], rows), val,
                            AF.Identity, bias=bias2[:, o : o + 1],
                        )
                    return
                # y2 = conv2(BN1(y1)) = ps + bias2 (scalar adds the bias)
                nc.scalar.activation(
                    _r3(y2[o][:, ysl], rows), val,
                    AF.Identity, bias=bias2[:, o : o + 1],
                )
                nc.vector.tensor_reduce(
                    st[:, jj : jj + 1], val, mybir.AxisListType.XY, ALU.add,
                )
                nc.scalar.activation(
                    _r3(sq[:, 0 : rows * 80], rows), val, AF.Square,
                    bias=bias2[:, o : o + 1],
                    accum_out=st[:, ncol + jj : ncol + jj + 1],
                )

            # BN2 apply + residual + store for one half; scalar scale-shifts
            # in place (no ACT penalty), DVE writes fresh tiles (in-place
            # penalty), pool takes early adds; per-chunk DMA on both
            # hardware DGE queues.
            def apply_half(o, scl, sh, hidden):
                fsls = [slice(1600 * jc, 1600 * (jc + 1)) for jc in range(4)]
                srcs = {}

                def ss(jc, eng):
                    fsl = fsls[jc]
                    if eng is nc.scalar:
                        nc.scalar.activation(
                            y2[o][:, fsl], y2[o][:, fsl], AF.Identity,
                            bias=sh[:, 0:1], scale=scl[:, 0:1],
                        )
                        srcs[jc] = y2[o][:, fsl]
                    else:
                        tmp = fin.tile([128, 1600], BF16, tag="tmp",
                                       name="tmp")
                        eng.tensor_scalar(
                            tmp[:, :], y2[o][:, fsl], scl[:, 0:1],
                            sh[:, 0:1], ALU.mult, ALU.add,
                        )
                        srcs[jc] = tmp[:, :]

                def add(jc, eng):
                    fsl = fsls[jc]
                    if eng is nc.gpsimd:
                        eng.tensor_tensor(
                            y2[o][:, fsl], srcs[jc], ftb[o][:, fsl], ALU.add
                        )
                        srcs[jc] = y2[o][:, fsl]
                    else:
                        ost = fin.tile([128, 1600], BF16, tag="ost",
                                       name="ost")
                        eng.tensor_tensor(
                            ost[:, :], srcs[jc], ftb[o][:, fsl], ALU.add
                        )
                        srcs[jc] = ost[:, :]

                def dma(jc, eng):
                    eng.dma_start(out_d[o * 128 : (o + 1) * 128, fsls[jc]],
                                  srcs[jc])

                ss(0, nc.scalar)
                ss(1, nc.scalar)
                add(0, nc.gpsimd)
                ss(2, nc.vector)
                add(2, nc.vector)
                dma(2, nc.sync)
                ss(3, nc.vector)
                add(3, nc.vector)
                dma(3, nc.sync)
                add(1, nc.gpsimd if hidden else nc.vector)
                dma(0, nc.sync)
                dma(1, nc.scalar)

            # phase o=0: all tiles, then a single fully-hidden all-reduce
            for j in range(NRT2):
                conv2_half(j, 0, st_o0, j, NRT2, False)
            ar2o0 = ar_chunk(st_o0, NRT2, 2, "2o0")
            bias_fix(ar2o0[0], 0, HW)
            ar_launch(*ar2o0)

            # phase o=1 with the o=0 coeffs / apply / output DMA woven in
            scl2a = sh2a = ar2A = None
            for j in range(NRT2):
                if j < N2A:
                    st, jj, ncol = stats2, j, N2A
                else:
                    st, jj, ncol = stats2b, j - N2A, NRT2 - N2A
                conv2_half(j, 1, st, jj, ncol, j >= N2A)
                if j == 1:
                    gst2a = ar_fetch(ar2o0[2], "2o0", ngrp=2)
                    scl2a, sh2a = bn_coeffs(gst2a, g2[:, 0:1], b2[:, 0:1],
                                            "2o0", no=1)
                elif j == 3:
                    apply_half(0, scl2a, sh2a, hidden=True)
                elif j == N2A - 1:
                    ar2A = ar_chunk(stats2, N2A, 2, "2a")
                    bias_fix(ar2A[0], 1, HW)
                    ar_launch(*ar2A)

            gst2b1 = ar_fetch(ar2A[2], "2a", ngrp=2)
            ar2B = ar_chunk(stats2b, NRT2 - N2A, 2, "2b")
            ar_launch(*ar2B)
            for o, ysl, rows, val in defer2:
                nc.vector.tensor_scalar(
                    _r3(y2[o][:, ysl], rows),
                    val, bias2[:, o : o + 1], None, ALU.add,
                )
            gst2b2 = ar_fetch(ar2B[2], "2b", ngrp=2)
            gst2 = small.tile([128, 2], F32, tag="gst2", name="gst2")
            nc.vector.tensor_tensor(gst2[:, :], gst2b1[:, :], gst2b2[:, :],
                                    ALU.add)
            scl2, sh2 = bn_coeffs(gst2, g2[:, 1:2], b2[:, 1:2], "2o1", no=1)
            apply_half(1, scl2, sh2, hidden=False)

    nc.compile()
    return nc


def _mblk():
    mb = np.zeros((8, 256), np.float32)
    for h in range(8):
        mb[h, h * 32 : (h + 1) * 32] = 1.0
    return mb.astype(ml_dtypes.bfloat16)


def _prep_inputs(feat0, zone_mask, w_q, w_k, w_v, conv1_w, bn1_g, bn1_b,
                 conv2_w, bn2_g, bn2_b, num_inside):
    B = feat0.shape[0]
    pos = np.asarray(zone_mask[:, :, 0])
    order = np.argsort(~pos, axis=1, kind="stable")
    assert np.array_equal(
        order[:, :num_inside],
        np.broadcast_to(np.arange(num_inside), (B, num_inside)),
    ), "kernel assumes inside positions are the first num_inside rows"
    assert num_inside == NI

    bf = ml_dtypes.bfloat16
    f32 = np.float32

    def wt(w):  # [dout, din] -> [128, 2, dout]: [p, ki, o] = w[o, ki*128+p]
        return np.ascontiguousarray(
            w.T.reshape(2, 128, D).transpose(1, 0, 2)
        ).astype(bf)

    def cw(w, nchunk):  # [O, I, 3, 3] -> [128, 9*nchunk, O]
        o_, i_, _, _ = w.shape
        r = w.transpose(2, 3, 1, 0).reshape(9, nchunk, 128, o_)
        return np.ascontiguousarray(
            r.transpose(2, 0, 1, 3).reshape(128, 9 * nchunk, o_)
        ).astype(bf)

    c2 = np.asarray(conv2_w, f32)
    # tap-summed conv2 weights for the folded-BN bias: [128, chunk, O]
    ws2 = np.ascontiguousarray(
        c2.sum(axis=(2, 3)).T.reshape(2, 128, D).transpose(1, 0, 2)
    ).astype(bf)

    common = {
        "wqt": wt(np.asarray(w_q, f32)),
        "wkt": wt(np.asarray(w_k, f32)),
        "wvt": wt(np.asarray(w_v, f32)),
        "c1w": cw(np.asarray(conv1_w, f32), 4),
        "c2w": cw(c2, 2),
        "ws2": ws2,
        "bn1g": np.asarray(bn1_g, f32).reshape(D, 1),
        "bn1b": np.asarray(bn1_b, f32).reshape(D, 1),
        "bn2g": np.asarray(bn2_g, f32).reshape(D, 1),
        "bn2b": np.asarray(bn2_b, f32).reshape(D, 1),
        "mblk": _mblk(),
    }
    in_maps = []
    for b in range(NCORES):
        m = dict(common)
        m["ft"] = np.ascontiguousarray(
            np.asarray(feat0[b], f32).T
        ).astype(bf)
        in_maps.append(m)
    return in_maps


def kernel(feat0, zone_mask, w_q, w_k, w_v, conv1_w, bn1_g, bn1_b,
           conv2_w, bn2_g, bn2_b, H=80, W=80, B=8, D=256, num_inside=3200,
           **_ignored):
    global LAST_EXEC_NS, LAST_MEAN_EXEC_NS
    if "nc" not in _cache:
        _cache["nc"] = build_nc()
    nc = _cache["nc"]

    in_maps = _prep_inputs(feat0, zone_mask, w_q, w_k, w_v, conv1_w, bn1_g,
                           bn1_b, conv2_w, bn2_g, bn2_b, int(num_inside))
    trace = os.environ.get("KERNEL_TRACE", "0") == "1"
    res = run_bass_kernel_spmd(nc, in_maps, list(range(NCORES)), trace=trace)
    LAST_EXEC_NS = res.exec_time_ns
    LAST_MEAN_EXEC_NS = res.mean_exec_time_ns
    out = np.empty((NCORES, HW, 256), np.float32)
    for b in range(NCORES):
        out[b] = np.asarray(res.results[b]["out_t"], np.float32).T
    return out



# revision 48
# speedup vs baseline: 1.0100x; 1.0100x over previous
"""Trainium2 Bass kernel for LoFTR-style encoder layer (sparse attention + convs).

Sharding: pure data-parallel over batch B=8 -> 8 NeuronCores (one batch
element per core). BN statistics are all-reduced across cores.

Schedule: ft is uploaded pre-cast to bf16 (halves the input DMA and
removes the on-device casts) with the first-needed slices leading both
DMA queues, so the K/V projections and conv1 start within ~10us;
conv1's row-tiles are interleaved with the vector-bound attention
phase (f-only tiles have no message dependency; later tiles follow one
message-tile behind) so the PE never starves. Each BN barrier uses a
two-chunk all-reduce: chunk A (most tiles) launches while the PE still
has tiles in flight and hides its full latency (including cross-core
drift); only chunk B's short latency is exposed. Two dummy warm-up
collectives absorb the CC cold-start during the attention window. BN1
is folded into conv2 (weights scaled by scl1 in o=0-first quarters,
bias from host-precomputed tap-summed weights, padding halo written as
-sh1/scl1 so zero-padding is reproduced exactly; halo writes split
vector/pool). conv2 is tiled at 6 rows per psum bank (fewer weight
loads); the last tile's y1p/y2 writes are deferred past the AR-B
launch. The BN2+residual tail runs on three engines concurrently
(scalar scale-shift in place, vector out-of-place, pool residual
adds), each chunk DMA-ing out immediately on the two hardware DGE
queues. Output is stored bf16 and converted on host.

Device layout is channel-major ([C, spatial]); host does the (free)
transposes / weight reordering / bf16 casts when staging inputs, and
transposes the per-core outputs back.
"""

import os
import sys

import numpy as np

for _p in ("/opt/trn_rl_repo", os.path.expanduser("~/.axon_site/_ro/trn_rl_repo")):
    if os.path.isdir(_p) and _p not in sys.path:
        sys.path.insert(0, _p)

import ml_dtypes

import concourse.bass as bass
import concourse.mybir as mybir
import concourse.tile as tile
from concourse import bacc
from concourse.bass_utils import run_bass_kernel_spmd

F32 = mybir.dt.float32
BF16 = mybir.dt.bfloat16
AF = mybir.ActivationFunctionType
ALU = mybir.AluOpType

NCORES = 8
H = W = 80
HW = H * W          # 6400
D = 256
NI = 3200           # inside positions (image rows 0..39)
NHEAD = 8
PW = W + 2          # 82 padded width
BN_EPS = 1e-5
BN_N = float(NCORES * HW)

# conv row-tiling: 5 output rows per psum tile -> N = 5*82 = 410 <= 512
RT = 5
NRT = H // RT       # 16
NT = RT * PW        # 410

LAST_EXEC_NS = None
LAST_MEAN_EXEC_NS = None

_cache = {}


def _bd(ap3):
    return ap3.rearrange("p a b -> p (a b)")


def _r3(ap2, a):
    return ap2.rearrange("p (a b) -> p a b", a=a)


def build_nc():
    nc = bacc.Bacc(
        "TRN2", target_bir_lowering=False, debug=False, num_devices=NCORES
    )

    ft_d = nc.dram_tensor("ft", [D, HW], BF16, kind="ExternalInput")
    wqt_d = nc.dram_tensor("wqt", [128, 2, D], BF16, kind="ExternalInput")
    wkt_d = nc.dram_tensor("wkt", [128, 2, D], BF16, kind="ExternalInput")
    wvt_d = nc.dram_tensor("wvt", [128, 2, D], BF16, kind="ExternalInput")
    c1w_d = nc.dram_tensor("c1w", [128, 36, D], BF16, kind="ExternalInput")
    c2w_d = nc.dram_tensor("c2w", [128, 18, D], BF16, kind="ExternalInput")
    ws2_d = nc.dram_tensor("ws2", [128, 2, D], BF16, kind="ExternalInput")
    bn1g_d = nc.dram_tensor("bn1g", [D, 1], F32, kind="ExternalInput")
    bn1b_d = nc.dram_tensor("bn1b", [D, 1], F32, kind="ExternalInput")
    bn2g_d = nc.dram_tensor("bn2g", [D, 1], F32, kind="ExternalInput")
    bn2b_d = nc.dram_tensor("bn2b", [D, 1], F32, kind="ExternalInput")
    mblk_d = nc.dram_tensor("mblk", [8, 256], BF16, kind="ExternalInput")
    out_d = nc.dram_tensor("out_t", [D, HW], BF16, kind="ExternalOutput")

    groups = [list(range(NCORES))]

    with tile.TileContext(nc) as tc:
        with (
            tc.tile_pool(name="pers", bufs=1) as pers,
            tc.tile_pool(name="bigp", bufs=2) as bigp,
            tc.tile_pool(name="ftio", bufs=2) as ftio,
            tc.tile_pool(name="qtp", bufs=4) as qtp,
            tc.tile_pool(name="scr", bufs=4) as scr,
            tc.tile_pool(name="small", bufs=1) as small,
            tc.tile_pool(name="fin", bufs=4) as fin,
            tc.tile_pool(name="psA", bufs=4, space="PSUM") as psA,
            tc.tile_pool(name="psS", bufs=2, space="PSUM") as psS,
            tc.tile_pool(name="psC", bufs=2, space="PSUM") as psC,
            tc.tile_pool(name="dram", bufs=1, space="DRAM") as dramp,
        ):
            # ---------------- weights / consts ------------------------------
            # ft arrives pre-cast to bf16 (host does the transpose+cast for
            # free), DMA'd straight into the persistent ftb tiles across four
            # trigger queues (sync/gpsimd for ft, scalar/vector for conv
            # weights) so the K/V matmuls and conv1 start as early as the DMA
            # engines allow.
            wqt = pers.tile([128, 2, D], BF16, tag="wqt", name="wqt")
            wkt = pers.tile([128, 2, D], BF16, tag="wkt", name="wkt")
            wvt = pers.tile([128, 2, D], BF16, tag="wvt", name="wvt")
            ftb = [
                pers.tile([128, HW], BF16, tag=f"ftb{m}", name=f"ftb{m}")
                for m in range(2)
            ]
            ip1 = [
                pers.tile([128, 84, PW], BF16, tag=f"ip1_{c}", name=f"ip1_{c}")
                for c in range(4)
            ]
            ip1f = [_bd(ip1[c][:, :, :]) for c in range(4)]
            c1w = pers.tile([128, 36, D], BF16, tag="c1w", name="c1w")
            c2w = pers.tile([128, 18, D], BF16, tag="c2w", name="c2w")
            ws2 = pers.tile([128, 2, D], BF16, tag="ws2", name="ws2")
            maskblk = pers.tile([8, 256], BF16, tag="maskblk", name="maskblk")

            # ft half 0 rides the sync hw-DGE queue; half 1 rides gpsimd's
            # software-DGE queue. gpsimd descriptor generation is slower
            # (~2.5us/transfer) but paces the HBM traffic deterministically —
            # with both hw queues blasting at once, per-core completion
            # jitter (~15us) lands in the exposed BN all-reduce waits.
            def ft_dma(m, c0, c1):
                eng = nc.sync if m == 0 else nc.gpsimd
                eng.dma_start(ftb[m][:, c0:c1],
                              ft_d[m * 128 : (m + 1) * 128, c0:c1])

            ft_dma(0, 0, 256)
            ft_dma(1, 0, 256)
            nc.sync.dma_start(wkt[:, :, :], wkt_d[:, :, :])
            nc.sync.dma_start(wvt[:, :, :], wvt_d[:, :, :])
            ft_dma(0, 256, 1600)
            ft_dma(1, 256, 1600)
            ft_dma(0, 1600, 3200)
            ft_dma(1, 1600, 3200)
            nc.sync.dma_start(wqt[:, :, :], wqt_d[:, :, :])
            nc.scalar.dma_start(c1w[:, 0:18, :], c1w_d[:, 0:18, :])
            nc.scalar.dma_start(c1w[:, 18:36, :], c1w_d[:, 18:36, :])
            ft_dma(0, 3200, 4800)
            ft_dma(1, 3200, 4800)
            ft_dma(0, 4800, 6400)
            ft_dma(1, 4800, 6400)
            nc.scalar.dma_start(c2w[:, :, :], c2w_d[:, :, :])
            nc.sync.dma_start(ws2[:, :, :], ws2_d[:, :, :])
            nc.sync.dma_start(maskblk[:, :], mblk_d[:, :])

            # warm up the CC cores with two dummy all-reduces: the first two
            # collective executions pay a ~20us cold-start which would
            # otherwise land in the BN1 bubble; here it overlaps attention.
            wrm = small.tile([8, 2], F32, tag="wrm", name="wrm")
            nc.vector.memset(wrm[:, :], 0.0)
            for w in range(2):
                win = dramp.tile([8, 2], F32, tag=f"win{w}", name=f"win{w}")
                wout = dramp.tile([8, 2], F32, tag=f"wout{w}", name=f"wout{w}")
                nc.sync.dma_start(win[:, :], wrm[:, :])
                nc.gpsimd.collective_compute(
                    "AllReduce", ALU.add, replica_groups=groups,
                    ins=[win[:, :].opt()], outs=[wout[:, :].opt()],
                )

            eps_t = small.tile([128, 1], F32, tag="eps_t", name="eps_t")
            nc.vector.memset(eps_t[:, :], BN_EPS)
            ones_t = small.tile([128, 164], F32, tag="ones_t", name="ones_t")
            nc.vector.memset(ones_t[:, :], 1.0)
            g1 = small.tile([128, 2], F32, tag="g1", name="g1")
            b1 = small.tile([128, 2], F32, tag="b1", name="b1")
            g2 = small.tile([128, 2], F32, tag="g2", name="g2")
            b2 = small.tile([128, 2], F32, tag="b2", name="b2")
            for o in range(2):
                sl = slice(o * 128, (o + 1) * 128)
                nc.sync.dma_start(g1[:, o : o + 1], bn1g_d[sl, :])
                nc.sync.dma_start(b1[:, o : o + 1], bn1b_d[sl, :])
                nc.sync.dma_start(g2[:, o : o + 1], bn2g_d[sl, :])
                nc.sync.dma_start(b2[:, o : o + 1], bn2b_d[sl, :])

            # ---------------- ip1 halo zeroing (targeted, not full tiles) ---
            # tile row r+2 == image row r; taps read tile rows 1..82 and the
            # 410-window spills touch rows 0 and 83 in discarded columns.
            for c in range(2):  # f-chunks
                nc.vector.memset(ip1[c][:, 0:2, :], 0.0)
                nc.vector.memset(ip1[c][:, 82:84, :], 0.0)
                nc.vector.memset(ip1[c][:, 2:82, 0:1], 0.0)
                nc.vector.memset(ip1[c][:, 2:82, 81:82], 0.0)
            for c in range(2, 4):  # t-chunks: zero band above message rows
                nc.vector.memset(ip1[c][:, 34:42, :], 0.0)
                nc.vector.memset(ip1[c][:, 82:84, :], 0.0)
                nc.vector.memset(ip1[c][:, 42:82, 0:1], 0.0)
                nc.vector.memset(ip1[c][:, 42:82, 81:82], 0.0)

            # ---------------- per-segment cast + ip1 interior + K/V proj ----
            ke = bigp.tile([128, 25, D], BF16, tag="big", name="ke")
            # ve layout [ones, v0..v255, ones]: per half m the 129 columns
            # [m*129 : m*129+129) are contiguous = [ones|v_m] or [v_m|ones]
            ve = bigp.tile([128, 25, D + 2], BF16, tag="big", name="ve")
            nc.vector.memset(ve[:, :, 0:1], 1.0)
            nc.vector.memset(ve[:, :, 257:258], 1.0)

            def kv_tile(i):
                ps = psA.tile([128, D], F32, tag="psA", name="psA")
                for ki in range(2):
                    nc.tensor.matmul(
                        ps[:, :],
                        ftb[ki][:, i * 128 : (i + 1) * 128],
                        wkt[:, ki, :],
                        start=(ki == 0),
                        stop=(ki == 1),
                    )
                # elu(x)+1 = relu(x) + exp(min(x,0))
                sm = scr.tile([128, 400], F32, tag="scr", name="sm")
                se = scr.tile([128, 400], F32, tag="scr", name="se")
                nc.vector.tensor_scalar_min(sm[:, :D], ps[:, :], 0.0)
                nc.scalar.activation(se[:, :D], sm[:, :D], AF.Exp)
                nc.vector.scalar_tensor_tensor(
                    ke[:, i, :], ps[:, :], 0.0, se[:, :D], ALU.max, ALU.add
                )
                ps2 = psA.tile([128, D], F32, tag="psA", name="psA")
                for ki in range(2):
                    nc.tensor.matmul(
                        ps2[:, :],
                        ftb[ki][:, i * 128 : (i + 1) * 128],
                        wvt[:, ki, :],
                        start=(ki == 0),
                        stop=(ki == 1),
                    )
                nc.vector.tensor_copy(ve[:, i, 1:257], ps2[:, :])

            def stage_seg(s, m):
                fseg = ftb[m][:, s * 1600 : (s + 1) * 1600]
                nc.vector.tensor_copy(
                    ip1[m][:, 2 + 20 * s : 22 + 20 * s, 1:81], _r3(fseg, 20)
                )

            kv_tile(0)
            kv_tile(1)
            for s in range(2):  # inside segments
                for m in range(2):
                    stage_seg(s, m)
                for i in range(12 * s + 2 * (1 - s), 12 * s + 12 + s):
                    kv_tile(i)  # s=0: 2..11, s=1: 12..24

            for s in range(2, 4):  # outside segments
                for m in range(2):
                    stage_seg(s, m)

            # ---------------- conv1 helpers ----------------
            y1p = [
                pers.tile([128, 84, PW], BF16, tag=f"y1p_{o}", name=f"y1p_{o}")
                for o in range(2)
            ]
            for o in range(2):  # spill-read guard rows (discarded columns)
                nc.vector.memset(y1p[o][:, 0:1, :], 0.0)
                nc.vector.memset(y1p[o][:, 83:84, :], 0.0)
            # BN1 stats are all-reduced in two chunks: chunk A (tiles 0..10)
            # launches mid-attention and hides its full latency under the
            # remaining conv1 tiles; only chunk B's short latency is exposed.
            N1A = 11
            stats1 = small.tile([128, 4 * N1A], F32, tag="stats1", name="stats1")
            stats1b = small.tile([128, 4 * (NRT - N1A)], F32, tag="stats1b",
                                 name="stats1b")

            def conv1_tile(j):
                r0 = RT * j
                if j < N1A:
                    st, jj, ncol = stats1, j, N1A
                else:
                    st, jj, ncol = stats1b, j - N1A, NRT - N1A
                fast = j >= 14  # last tiles: stats first, y1p copy on vector
                taps = []
                for c in range(4):
                    for ky in range(3):
                        if c >= 2 and r0 + ky + 4 < 41:
                            continue  # t-channel rows all zero
                        for kx in range(3):
                            taps.append((c, ky, kx))
                for o in range(2):
                    ps = psC.tile([128, NT], F32, tag="psC", name="psC")
                    for idx, (c, ky, kx) in enumerate(taps):
                        s = (r0 + ky + 1) * PW + kx - 1
                        nc.tensor.matmul(
                            ps[:, :],
                            c1w[:, (ky * 3 + kx) * 4 + c, o * 128 : (o + 1) * 128],
                            ip1f[c][:, s : s + NT],
                            start=(idx == 0),
                            stop=(idx == len(taps) - 1),
                        )
                    val = _r3(ps[:, :], RT)[:, :, 1:81]
                    sq = scr.tile([128, 400], F32, tag="scr", name="sq")
                    if fast:
                        nc.scalar.activation(
                            _r3(sq[:, :], RT), val, AF.Square,
                            accum_out=st[:, (o * 2 + 1) * ncol + jj :
                                         (o * 2 + 1) * ncol + jj + 1],
                        )
                        nc.vector.tensor_reduce(
                            st[:, (o * 2) * ncol + jj : (o * 2) * ncol + jj + 1],
                            val, mybir.AxisListType.XY, ALU.add,
                        )
                        if j == NRT - 1:
                            # last tile: y1p copy deferred past the AR-B
                            # launch so the collective triggers ASAP
                            defer1.append((o, r0, val))
                        else:
                            # scalar (after the Square) keeps the vector
                            # queue clear for the AR-B reduce
                            nc.scalar.copy(
                                y1p[o][:, 2 + r0 : 7 + r0, 1:81], val
                            )
                        continue
                    nc.scalar.copy(
                        y1p[o][:, 2 + r0 : 7 + r0, 1:81], val
                    )
                    nc.vector.tensor_reduce(
                        st[:, (o * 2) * ncol + jj : (o * 2) * ncol + jj + 1],
                        val, mybir.AxisListType.XY, ALU.add,
                    )
                    nc.scalar.activation(
                        _r3(sq[:, :], RT), val, AF.Square,
                        accum_out=st[:, (o * 2 + 1) * ncol + jj :
                                     (o * 2 + 1) * ncol + jj + 1],
                    )

            # f-only conv1 tiles 0,1 ahead of the KV->bd accumulation:
            # the bd chain waits on per-i scalar ve copies, so give the PE
            # filler work to absorb that lag.
            conv1_tile(0)
            conv1_tile(1)

            # ---------------- KV + Ksum -> block-diag BD ----------------
            bd = [
                pers.tile([128, 136], BF16, tag=f"bd{m}", name=f"bd{m}")
                for m in range(2)
            ]
            for m in range(2):
                psm = psA.tile([128, 129], F32, tag="psA", name="psA")
                for i in range(25):
                    nc.tensor.matmul(
                        psm[:, :],
                        ke[:, i, m * 128 : (m + 1) * 128],
                        ve[:, i, m * 129 : m * 129 + 129],
                        start=(i == 0),
                        stop=(i == 24),
                    )
                kcol = 0 if m == 0 else 128
                voff = 1 - m
                nc.vector.memset(bd[m][:, :], 0.0)
                for hh in range(4):
                    h = m * 4 + hh
                    lh = hh * 32
                    nc.vector.tensor_copy(
                        bd[m][lh : lh + 32, lh : lh + 32],
                        psm[lh : lh + 32, voff + lh : voff + lh + 32],
                    )
                    nc.vector.tensor_copy(
                        bd[m][lh : lh + 32, 128 + h : 129 + h],
                        psm[lh : lh + 32, kcol : kcol + 1],
                    )

            # ---------- Q/message pipeline interleaved with f-only conv1 ----
            def qproj(j):
                lsl = slice(NI + j * 400, NI + (j + 1) * 400)
                qt = [
                    qtp.tile([128, 400], BF16, tag="qteT", name=f"qt{m}")
                    for m in range(2)
                ]
                qps = []
                for m in range(2):
                    ps = psA.tile([128, 400], F32, tag="psA", name="psA")
                    for ki in range(2):
                        nc.tensor.matmul(
                            ps[:, :],
                            wqt[:, ki, m * 128 : (m + 1) * 128],
                            ftb[ki][:, lsl],
                            start=(ki == 0),
                            stop=(ki == 1),
                        )
                    qps.append(ps)
                return qt, qps

            def qelu(qt, qps):
                for m in range(2):
                    sm = scr.tile([128, 400], F32, tag="scr", name="smq")
                    se = scr.tile([128, 400], F32, tag="scr", name="seq")
                    nc.vector.tensor_scalar_min(sm[:, :], qps[m][:, :], 0.0)
                    nc.scalar.activation(se[:, :], sm[:, :], AF.Exp)
                    nc.vector.scalar_tensor_tensor(
                        qt[m][:, :], qps[m][:, :], 0.0, se[:, :], ALU.max, ALU.add
                    )

            def qmessage(j, qt):
                pss = psS.tile([8, 400], F32, tag="psS", name="psS")
                for ki in range(2):
                    nc.tensor.matmul(
                        pss[:, :],
                        bd[ki][:, 128:136],
                        qt[ki][:, :],
                        start=(ki == 0),
                        stop=(ki == 1),
                    )
                # S >> eps (S >= ~1e2), so 1/(S+eps) == 1/S in fp32
                rsf = scr.tile([128, 400], F32, tag="scr", name="rsf")
                rs = scr.tile([128, 400], BF16, tag="scr", name="rs")
                nc.vector.reciprocal_approx_fast(rsf[:8, :], pss[:, :])
                nc.scalar.copy(rs[:8, :], rsf[:8, :])

                for m in range(2):
                    psg = psA.tile([128, 400], F32, tag="psA", name="psA")
                    nc.tensor.matmul(
                        psg[:, :], bd[m][:, 0:128], qt[m][:, :],
                        start=True, stop=True,
                    )
                    pre = psS.tile([128, 400], F32, tag="psS", name="psS")
                    nc.tensor.matmul(
                        pre[:, :], maskblk[:, m * 128 : (m + 1) * 128], rs[:8, :]
                    )
                    preb = scr.tile([128, 400], BF16, tag="scr", name="preb")
                    nc.scalar.copy(preb[:, :], pre[:, :])
                    # l-tile j = image rows 40+5j..44+5j -> tile rows 42+5j..
                    nc.vector.tensor_tensor(
                        ip1[2 + m][:, 42 + 5 * j : 47 + 5 * j, 1:81],
                        _r3(psg[:, :], RT),
                        _r3(preb[:, :], RT),
                        ALU.mult,
                    )

            def ar_chunk(stats, ncol, ngrp, tag):
                sv = stats[:, :].rearrange("p (k j) -> p k j", j=ncol)
                bnst = small.tile([128, ngrp], F32, tag=f"bnst{tag}",
                                  name=f"bnst{tag}")
                arin = dramp.tile([128, ngrp], F32, tag=f"arin{tag}",
                                  name=f"arin{tag}")
                arout = dramp.tile([128, ngrp], F32, tag=f"arout{tag}",
                                   name=f"arout{tag}")
                nc.vector.tensor_reduce(
                    bnst[:, :], sv[:, :, :], mybir.AxisListType.X, ALU.add
                )
                return bnst, arin, arout

            def ar_launch(bnst, arin, arout):
                nc.sync.dma_start(arin[:, :], bnst[:, :])
                nc.gpsimd.collective_compute(
                    "AllReduce", ALU.add, replica_groups=groups,
                    ins=[arin[:, :].opt()], outs=[arout[:, :].opt()],
                )

            def ar_fetch(arout, tag, ngrp=4):
                g = small.tile([128, ngrp], F32, tag=f"gst{tag}",
                               name=f"gst{tag}")
                nc.sync.dma_start(g[:, :], arout[:, :])
                return g

            # software pipeline: Qproj j+1 is issued before the dependent
            # attention tail of j; f-only conv1 tiles fill the PE while the
            # vector engine runs elu/Z.
            ar1A = None
            qt_c, qps_c = qproj(0)
            for j in range(8):
                qelu(qt_c, qps_c)
                if j <= 4:
                    conv1_tile(j + 2)   # f-only tiles (t rows all zero)
                nxt = qproj(j + 1) if j < 7 else None
                qmessage(j, qt_c)
                if j >= 1:
                    # tile j+6 needs message j-1 (just written): keeps the PE
                    # queue deep so the elu chain latency never starves it
                    conv1_tile(j + 6)
                if j == 4:
                    # stats chunk A (tiles 0..10) all-reduce: ~60us of PE
                    # work remains, so its full latency (incl. core skew)
                    # hides under conv1.
                    ar1A = ar_chunk(stats1, N1A, 4, "1a")
                    ar_launch(*ar1A)
                if nxt is not None:
                    qt_c, qps_c = nxt
            gst1a = ar_fetch(ar1A[2], "1a")

            # pre-warm the sqrt activation table now that the last Exp is
            # issued: the 1.28us table switch hides under conv1 instead of
            # landing in the BN1 critical path.
            sqwrm = small.tile([128, 1], F32, tag="sqwrm", name="sqwrm")
            nc.scalar.activation(sqwrm[:, :], eps_t[:, :], AF.Sqrt)

            defer1 = []
            for j in range(14, NRT):
                conv1_tile(j)
            ar1B = ar_chunk(stats1b, NRT - N1A, 4, "1b")
            ar_launch(*ar1B)
            # the deferred copies drain on the idle vector queue during the
            # AR-B wait, well before conv2's first psum reuses their banks
            for o, r0, val in defer1:
                nc.vector.tensor_copy(y1p[o][:, 2 + r0 : 7 + r0, 1:81], val)
            gst1b = ar_fetch(ar1B[2], "1b")
            gst1 = small.tile([128, 4], F32, tag="gst1", name="gst1")
            nc.vector.tensor_tensor(gst1[:, :], gst1a[:, :], gst1b[:, :],
                                    ALU.add)

            def bn_coeffs(gst, gg, bb, tag, no=2):
                nm = small.tile([128, no], F32, tag=f"nm{tag}", name=f"nm{tag}")
                ex2 = small.tile([128, no], F32, tag=f"ex2{tag}", name=f"ex2{tag}")
                var = small.tile([128, no], F32, tag=f"var{tag}", name=f"var{tag}")
                sd = small.tile([128, no], F32, tag=f"sd{tag}", name=f"sd{tag}")
                rsd = small.tile([128, no], F32, tag=f"rsd{tag}", name=f"rsd{tag}")
                scl = small.tile([128, no], F32, tag=f"scl{tag}", name=f"scl{tag}")
                sh = small.tile([128, no], F32, tag=f"sh{tag}", name=f"sh{tag}")
                gv = gst[:, :].rearrange("p (o k) -> p k o", k=2)
                nc.vector.tensor_scalar_mul(nm[:, :], gv[:, 0, :], -1.0 / BN_N)
                nc.vector.tensor_scalar_mul(ex2[:, :], gv[:, 1, :], 1.0 / BN_N)
                # var_neg = m^2 - E[x^2];  sd = sqrt(-var_neg + eps)
                nc.vector.tensor_tensor(var[:, :], nm[:, :], nm[:, :], ALU.mult)
                nc.vector.tensor_tensor(
                    var[:, :], var[:, :], ex2[:, :], ALU.subtract
                )
                nc.scalar.activation(
                    sd[:, :], var[:, :], AF.Sqrt, bias=eps_t[:, 0:1], scale=-1.0
                )
                nc.vector.reciprocal(rsd[:, :], sd[:, :])
                nc.vector.tensor_tensor(scl[:, :], rsd[:, :], gg[:, :], ALU.mult)
                nc.vector.tensor_tensor(sh[:, :], nm[:, :], scl[:, :], ALU.mult)
                nc.vector.tensor_tensor(sh[:, :], sh[:, :], bb[:, :], ALU.add)
                return scl, sh

            scl1, sh1 = bn_coeffs(gst1, g1, b1, "1")

            # fold BN1 into conv2: w2' = w2 * scl1[c]; halo = -sh1/scl1 so
            # zero-padding maps to BN-output zero; bias2[o] = sum_{c,k} w2*sh1
            # The o=0 weight quarters are scaled first (conv2 tile 0 starts
            # on them) and the y1p[1] halo writes run on the pool engine in
            # parallel with vector's y1p[0] halos.
            hv1 = small.tile([128, 2], F32, tag="hv1", name="hv1")
            rscl = small.tile([128, 2], F32, tag="rscl", name="rscl")
            nc.vector.reciprocal(rscl[:, :], scl1[:, :])
            nc.vector.scalar_tensor_tensor(
                hv1[:, :], sh1[:, :], -1.0, rscl[:, :], ALU.mult, ALU.mult
            )

            def halos(o, eng):
                hvo = hv1[:, o : o + 1]
                eng.tensor_scalar(
                    y1p[o][:, 1:2, :], _r3(ones_t[:, 0:82], 1), hvo, None,
                    ALU.mult,
                )
                eng.tensor_scalar(
                    y1p[o][:, 82:83, :], _r3(ones_t[:, 0:82], 1), hvo, None,
                    ALU.mult,
                )
                eng.tensor_scalar(
                    y1p[o][:, 2:82, 0:1], _r3(ones_t[:, 0:80], 80), hvo, None,
                    ALU.mult,
                )
                eng.tensor_scalar(
                    y1p[o][:, 2:82, 81:82], _r3(ones_t[:, 0:80], 80), hvo,
                    None, ALU.mult,
                )

            halos(1, nc.gpsimd)
            c2wv = c2w[:, :, :].rearrange("p (t c) o -> p t c o", c=2)
            for oh in range(2):
                if oh == 1:
                    halos(0, nc.vector)
                for ck in range(2):
                    nc.vector.tensor_scalar(
                        c2wv[:, :, ck, oh * 128 : (oh + 1) * 128],
                        c2wv[:, :, ck, oh * 128 : (oh + 1) * 128],
                        scl1[:, ck : ck + 1], None, ALU.mult,
                    )


            bias2 = small.tile([128, 2], F32, tag="bias2", name="bias2")
            sh1b = small.tile([128, 2], BF16, tag="sh1b", name="sh1b")
            nc.scalar.copy(sh1b[:, :], sh1[:, :])
            for o in range(2):
                psb = psC.tile([128, NT], F32, tag="psC", name="psC")
                for ck in range(2):
                    nc.tensor.matmul(
                        psb[:, 0:1],
                        ws2[:, ck, o * 128 : (o + 1) * 128],
                        sh1b[:, ck : ck + 1],
                        start=(ck == 0),
                        stop=(ck == 1),
                    )
                nc.scalar.copy(bias2[:, o : o + 1], psb[:, 0:1])

            # ---------------- conv2 (+ stats), o-phased ----------------
            # The two output-channel halves have independent BN statistics:
            # all o=0 tiles run first, so their all-reduce, coefficients,
            # BN-apply, residual add and output DMA all hide under the o=1
            # tile compute; only the o=1 half's chunk-B collective and
            # 4-chunk apply are exposed at the end. Tiled at 6 rows per
            # psum bank (13 full tiles + one 2-row tail).
            y2 = [
                bigp.tile([128, HW], BF16, tag="big", name=f"y2_{o}")
                for o in range(2)
            ]
            RT2 = 6
            NRT2 = 14
            N2A = 11
            st_o0 = small.tile([128, 2 * NRT2], F32, tag="st_o0", name="st_o0")
            stats2 = small.tile([128, 2 * N2A], F32, tag="stats2", name="stats2")
            stats2b = small.tile([128, 2 * (NRT2 - N2A)], F32, tag="stats2b",
                                 name="stats2b")
            y1pf = [_bd(y1p[c][:, :, :]) for c in range(2)]

            def bias_fix(bnst, o, npos):
                # reduce() summed raw psum values; the true sums need
                # +npos*bias2 (the sq stats were already biased); rides a
                # hidden chunk, off the critical path.
                nc.vector.scalar_tensor_tensor(
                    bnst[:, 0:1], bias2[:, o : o + 1],
                    float(npos), bnst[:, 0:1], ALU.mult, ALU.add
                )

            defer2 = []

            def conv2_half(j, o, st, jj, ncol, fast):
                r0 = RT2 * j
                rows = RT2 if j < NRT2 - 1 else H - RT2 * (NRT2 - 1)
                nt = rows * PW
                ysl = slice(r0 * 80, (r0 + rows) * 80)
                ps = psC.tile([128, 492], F32, tag="psC", name="psC")
                idx = 0
                for c in range(2):
                    for ky in range(3):
                        for kx in range(3):
                            s = (r0 + ky + 1) * PW + kx - 1
                            nc.tensor.matmul(
                                ps[:, 0:nt],
                                c2w[:, (ky * 3 + kx) * 2 + c,
                                    o * 128 : (o + 1) * 128],
                                y1pf[c][:, s : s + nt],
                                start=(idx == 0),
                                stop=(idx == 17),
                            )
                            idx += 1
                val = _r3(ps[:, 0:nt], rows)[:, :, 1:81]
                sq = scr.tile([128, 496], F32, tag="scr2", name="sq2")
                if fast:
                    # stats first; the y2 write trails on scalar (or defers
                    # past the AR-B launch for the last tile)
                    nc.scalar.activation(
                        _r3(sq[:, 0 : rows * 80], rows), val, AF.Square,
                        bias=bias2[:, o : o + 1],
                        accum_out=st[:, ncol + jj : ncol + jj + 1],
                    )
                    nc.vector.tensor_reduce(
                        st[:, jj : jj + 1], val,
                        mybir.AxisListType.XY, ALU.add,
                    )
                    if j == NRT2 - 1:
                        defer2.append((o, ysl, rows, val))
                    else:
                        nc.scalar.activation(
                            _r3(y2[o][:, ysl], rows), val,
                            AF.Identity, bias=bias2[:, o : o + 1],
                        )
                    return
                # y2 = conv2(BN1(y1)) = ps + bias2 (scalar adds the bias)
                nc.scalar.activation(
                    _r3(y2[o][:, ysl], rows), val,
                    AF.Identity, bias=bias2[:, o : o + 1],
                )
                nc.vector.tensor_reduce(
                    st[:, jj : jj + 1], val, mybir.AxisListType.XY, ALU.add,
                )
                nc.scalar.activation(
                    _r3(sq[:, 0 : rows * 80], rows), val, AF.Square,
                    bias=bias2[:, o : o + 1],
                    accum_out=st[:, ncol + jj : ncol + jj + 1],
                )

            # BN2 apply + residual + store for one half; scalar scale-shifts
            # in place (no ACT penalty), DVE writes fresh tiles (in-place
            # penalty), pool takes early adds; per-chunk DMA on both
            # hardware DGE queues.
            def apply_half(o, scl, sh, hidden):
                fsls = [slice(1600 * jc, 1600 * (jc + 1)) for jc in range(4)]
                srcs = {}

                def ss(jc, eng):
                    fsl = fsls[jc]
                    if eng is nc.scalar:
                        nc.scalar.activation(
                            y2[o][:, fsl], y2[o][:, fsl], AF.Identity,
                            bias=sh[:, 0:1], scale=scl[:, 0:1],
                        )
                        srcs[jc] = y2[o][:, fsl]
                    else:
                        tmp = fin.tile([128, 1600], BF16, tag="tmp",
                                       name="tmp")
                        eng.tensor_scalar(
                            tmp[:, :], y2[o][:, fsl], scl[:, 0:1],
                            sh[:, 0:1], ALU.mult, ALU.add,
                        )
                        srcs[jc] = tmp[:, :]

                def add(jc, eng):
                    fsl = fsls[jc]
                    if eng is nc.gpsimd:
                        eng.tensor_tensor(
                            y2[o][:, fsl], srcs[jc], ftb[o][:, fsl], ALU.add
                        )
                        srcs[jc] = y2[o][:, fsl]
                    else:
                        ost = fin.tile([128, 1600], BF16, tag="ost",
                                       name="ost")
                        eng.tensor_tensor(
                            ost[:, :], srcs[jc], ftb[o][:, fsl], ALU.add
                        )
                        srcs[jc] = ost[:, :]

                def dma(jc, eng):
                    eng.dma_start(out_d[o * 128 : (o + 1) * 128, fsls[jc]],
                                  srcs[jc])

                ss(0, nc.scalar)
                ss(1, nc.scalar)
                add(0, nc.gpsimd)
                ss(2, nc.vector)
                add(2, nc.vector)
                dma(2, nc.sync)
                ss(3, nc.vector)
                add(3, nc.vector)
                dma(3, nc.sync)
                add(1, nc.gpsimd if hidden else nc.vector)
                dma(0, nc.sync)
                # hidden half: keep the scalar queue free of triggers that
                # wait on the slow pool add (head-of-line blocking)
                dma(1, nc.sync if hidden else nc.scalar)

            # phase o=0: all tiles, then a single fully-hidden all-reduce
            for j in range(NRT2):
                conv2_half(j, 0, st_o0, j, NRT2, False)
            ar2o0 = ar_chunk(st_o0, NRT2, 2, "2o0")
            bias_fix(ar2o0[0], 0, HW)
            ar_launch(*ar2o0)

            # phase o=1 with the o=0 coeffs / apply / output DMA woven in.
            # All o=0 work is issued only AFTER the chunk-A launch: issuing
            # it earlier head-of-line-blocks the in-order vector/scalar
            # queues on the AR2o0 fetch, starving the phase-o1 stats path.
            ar2A = None
            for j in range(NRT2):
                if j < N2A:
                    st, jj, ncol = stats2, j, N2A
                else:
                    st, jj, ncol = stats2b, j - N2A, NRT2 - N2A
                conv2_half(j, 1, st, jj, ncol, j >= N2A)
                if j == N2A - 1:
                    ar2A = ar_chunk(stats2, N2A, 2, "2a")
                    bias_fix(ar2A[0], 1, HW)
                    ar_launch(*ar2A)
                    # o=0 stats arrived long ago; coeffs + apply + store for
                    # that half drain under the remaining o=1 tiles
                    gst2a = ar_fetch(ar2o0[2], "2o0", ngrp=2)
                    scl2a, sh2a = bn_coeffs(gst2a, g2[:, 0:1], b2[:, 0:1],
                                            "2o0", no=1)
                    apply_half(0, scl2a, sh2a, hidden=True)

            gst2b1 = ar_fetch(ar2A[2], "2a", ngrp=2)
            ar2B = ar_chunk(stats2b, NRT2 - N2A, 2, "2b")
            ar_launch(*ar2B)
            for o, ysl, rows, val in defer2:
                nc.vector.tensor_scalar(
                    _r3(y2[o][:, ysl], rows),
                    val, bias2[:, o : o + 1], None, ALU.add,
                )
            gst2b2 = ar_fetch(ar2B[2], "2b", ngrp=2)
            gst2 = small.tile([128, 2], F32, tag="gst2", name="gst2")
            nc.vector.tensor_tensor(gst2[:, :], gst2b1[:, :], gst2b2[:, :],
                                    ALU.add)
            scl2, sh2 = bn_coeffs(gst2, g2[:, 1:2], b2[:, 1:2], "2o1", no=1)
            apply_half(1, scl2, sh2, hidden=False)

    nc.compile()
    return nc


def _mblk():
    mb = np.zeros((8, 256), np.float32)
    for h in range(8):
        mb[h, h * 32 : (h + 1) * 32] = 1.0
    return mb.astype(ml_dtypes.bfloat16)


def _prep_inputs(feat0, zone_mask, w_q, w_k, w_v, conv1_w, bn1_g, bn1_b,
                 conv2_w, bn2_g, bn2_b, num_inside):
    B = feat0.shape[0]
    pos = np.asarray(zone_mask[:, :, 0])
    order = np.argsort(~pos, axis=1, kind="stable")
    assert np.array_equal(
        order[:, :num_inside],
        np.broadcast_to(np.arange(num_inside), (B, num_inside)),
    ), "kernel assumes inside positions are the first num_inside rows"
    assert num_inside == NI

    bf = ml_dtypes.bfloat16
    f32 = np.float32

    def wt(w):  # [dout, din] -> [128, 2, dout]: [p, ki, o] = w[o, ki*128+p]
        return np.ascontiguousarray(
            w.T.reshape(2, 128, D).transpose(1, 0, 2)
        ).astype(bf)

    def cw(w, nchunk):  # [O, I, 3, 3] -> [128, 9*nchunk, O]
        o_, i_, _, _ = w.shape
        r = w.transpose(2, 3, 1, 0).reshape(9, nchunk, 128, o_)
        return np.ascontiguousarray(
            r.transpose(2, 0, 1, 3).reshape(128, 9 * nchunk, o_)
        ).astype(bf)

    c2 = np.asarray(conv2_w, f32)
    # tap-summed conv2 weights for the folded-BN bias: [128, chunk, O]
    ws2 = np.ascontiguousarray(
        c2.sum(axis=(2, 3)).T.reshape(2, 128, D).transpose(1, 0, 2)
    ).astype(bf)

    common = {
        "wqt": wt(np.asarray(w_q, f32)),
        "wkt": wt(np.asarray(w_k, f32)),
        "wvt": wt(np.asarray(w_v, f32)),
        "c1w": cw(np.asarray(conv1_w, f32), 4),
        "c2w": cw(c2, 2),
        "ws2": ws2,
        "bn1g": np.asarray(bn1_g, f32).reshape(D, 1),
        "bn1b": np.asarray(bn1_b, f32).reshape(D, 1),
        "bn2g": np.asarray(bn2_g, f32).reshape(D, 1),
        "bn2b": np.asarray(bn2_b, f32).reshape(D, 1),
        "mblk": _mblk(),
    }
    in_maps = []
    for b in range(NCORES):
        m = dict(common)
        m["ft"] = np.ascontiguousarray(
            np.asarray(feat0[b], f32).T
        ).astype(bf)
        in_maps.append(m)
    return in_maps


def kernel(feat0, zone_mask, w_q, w_k, w_v, conv1_w, bn1_g, bn1_b,
           conv2_w, bn2_g, bn2_b, H=80, W=80, B=8, D=256, num_inside=3200,
           **_ignored):
    global LAST_EXEC_NS, LAST_MEAN_EXEC_NS
    if "nc" not in _cache:
        _cache["nc"] = build_nc()
    nc = _cache["nc"]

    in_maps = _prep_inputs(feat0, zone_mask, w_q, w_k, w_v, conv1_w, bn1_g,
                           bn1_b, conv2_w, bn2_g, bn2_b, int(num_inside))
    trace = os.environ.get("KERNEL_TRACE", "0") == "1"
    res = run_bass_kernel_spmd(nc, in_maps, list(range(NCORES)), trace=trace)
    LAST_EXEC_NS = res.exec_time_ns
    LAST_MEAN_EXEC_NS = res.mean_exec_time_ns
    out = np.empty((NCORES, HW, 256), np.float32)
    for b in range(NCORES):
        out[b] = np.asarray(res.results[b]["out_t"], np.float32).T
    return out

